# revision 1
# baseline (speedup 1.0000x reference)
"""MoE layer (7 routed experts top-2 + 1 shared, D=1024, H=1024) on 8 trn2 cores.

Sharding: data-parallel over tokens (8192 tokens -> 1024/core), all weights
replicated. Per core, everything is computed feature-major ([feature, token]
layout) so weights load directly as matmul stationary operands and no on-chip
transposes are needed. Router runs in fp32 (top-k selection must match the
fp32 reference bit-for-bit in ordering); expert/shared matmuls run in fp32r
(full PE rate, ~1.4e-4 rel err). Host only slices/transposes/concats.

Compiled with walrus --enable-ldw-opt=true: consecutive matmuls sharing a
stationary operand (our two 512-token blocks per weight tile) dedup the
weight reload — verified to drop 1045 PE instructions (~110us) with
bit-identical output vs the default flag.
"""

import os
import sys

for _p in ("/opt/trn_rl_repo", "/root/.axon_site/_ro/trn_rl_repo"):
    if os.path.isdir(_p) and _p not in sys.path:
        sys.path.append(_p)

import numpy as np

import concourse.bacc as bacc
import concourse.bass as bass
import concourse.mybir as mybir
import concourse.tile as tile
from concourse import bass_utils
from concourse.masks import make_identity as masks_make_identity

# Patch the walrus invocation to honor a module-level LDW-opt switch.
# (--enable-ldw-opt lets walrus dedup back-to-back LDWEIGHTS for matmuls that
# share a stationary operand; concourse disables it by default.)
_LDW_OPT = False
_orig_run_command = bass_utils.run_command


def _run_command_ldw(argv, **kw):
    if _LDW_OPT:
        argv = ["--enable-ldw-opt=true" if a == "--enable-ldw-opt=false" else a
                for a in argv]
    return _orig_run_command(argv, **kw)


bass_utils.run_command = _run_command_ldw

# Problem constants (hardcoded per contract)
B, S, D, H = 4, 2048, 1024, 1024
E = 7            # routed experts
N_CORES = 8
T = B * S        # 8192 tokens
TC = T // N_CORES  # 1024 tokens per core
P = 128
DC = D // P      # 8 d-chunks
HC = H // P      # 8 h-chunks
NTB = 2          # token blocks of 512 (matmul moving-N)
TB = TC // NTB   # 512

F32 = mybir.dt.float32
F32R = mybir.dt.float32r


def build_nc(n_reps: int = 1, act_fn=None, mm_mode: str = "f32r", aux_f32r: bool = True,
             ldw_opt: bool = False):
    """Build the Bass program. n_reps>1 wraps the whole kernel in a hardware
    loop for timing purposes (test harness only). act_fn overrides the L1
    activation (CoreSim has no Gelu; tests substitute Sigmoid). mm_mode picks
    the expert-matmul dtype: "f32r" (tf32-like, self-loading MMs) or "f16"
    (half-precision weights/activations, separate overlapped LDWEIGHTS)."""
    if act_fn is None:
        act_fn = mybir.ActivationFunctionType.Gelu
    MMDT = {"f32r": F32R, "f16": mybir.dt.float16, "bf16": mybir.dt.bfloat16}[mm_mode]
    AUXDT = F32R if aux_f32r else F32
    nc = bacc.Bacc("TRN2", target_bir_lowering=False, debug=False)

    # ---- DRAM I/O (per-core shapes) ----
    xT = nc.dram_tensor("xT", (D, TC), F32, kind="ExternalInput")        # x shard, transposed
    gwT = nc.dram_tensor("gwT", (D, E), F32, kind="ExternalInput")       # gate_w.T
    eb = nc.dram_tensor("eb", (E,), F32, kind="ExternalInput")
    rev7 = nc.dram_tensor("rev7", (E,), F32, kind="ExternalInput")       # [7,6,...,1]
    sw1 = nc.dram_tensor("sw1", (D, H), MMDT, kind="ExternalInput")
    sb1 = nc.dram_tensor("sb1", (H,), F32, kind="ExternalInput")
    sw2 = nc.dram_tensor("sw2", (H, D), MMDT, kind="ExternalInput")
    sb2 = nc.dram_tensor("sb2", (D,), F32, kind="ExternalInput")
    rw1 = nc.dram_tensor("rw1", (E, D, H), MMDT, kind="ExternalInput")
    rb1 = nc.dram_tensor("rb1", (E, H), F32, kind="ExternalInput")
    rw2 = nc.dram_tensor("rw2", (E, H, D), MMDT, kind="ExternalInput")
    rb2 = nc.dram_tensor("rb2", (E, D), F32, kind="ExternalInput")
    outT = nc.dram_tensor("outT", (D, TC), F32, kind="ExternalOutput")

    xT_t = xT.rearrange("(c p) t -> p c t", p=P)       # [128, 8, 1024]
    gwT_t = gwT.rearrange("(c p) e -> p c e", p=P)     # [128, 8, 7]
    sb1_t = sb1.rearrange("(c p) -> p c", p=P)         # [128, 8]
    sb2_t = sb2.rearrange("(c p) -> p c", p=P)
    rb1_t = rb1.rearrange("e (c p) -> p e c", p=P)     # [128, 7, 8]
    outT_t = outT.rearrange("(c p) t -> p c t", p=P)

    def w1_view(n):
        # [d, h] layout -> [128, dc, h]
        if n == 0:
            return sw1.rearrange("(c p) h -> p c h", p=P)
        return rw1[n - 1].rearrange("(c p) h -> p c h", p=P)

    def w2_view(n):
        if n == 0:
            return sw2.rearrange("(c p) d -> p c d", p=P)
        return rw2[n - 1].rearrange("(c p) d -> p c d", p=P)

    with tile.TileContext(nc) as tc:
        with (
            tc.tile_pool(name="const", bufs=1) as constp,
            tc.tile_pool(name="xr", bufs=1) as xrp,
            tc.tile_pool(name="xchunk", bufs=2) as xcp,
            tc.tile_pool(name="w", bufs=4) as wp,
            tc.tile_pool(name="h", bufs=1) as hp,
            tc.tile_pool(name="acc", bufs=1) as accp,
            tc.tile_pool(name="bc", bufs=1) as bcp,
            tc.tile_pool(name="rt", bufs=1) as rtp,
            tc.tile_pool(name="psl", bufs=1, space="PSUM") as pslp,
            tc.tile_pool(name="ps", bufs=3, space="PSUM") as psp,
        ):
            def body(_iv=None):
                # ---------- constants / small tiles ----------
                gw_sb = constp.tile([P, DC, E], F32, tag="gw")
                nc.sync.dma_start(gw_sb[:], gwT_t[:])
                eb_sb = constp.tile([E, 1], F32, tag="eb")
                nc.sync.dma_start(eb_sb[:], eb[:, None])
                rev_sb = constp.tile([P, E], F32, tag="rev")
                nc.sync.dma_start(rev_sb[:], rev7[None, :].to_broadcast((P, E)))
                ident = constp.tile([P, P], F32, tag="ident")
                masks_make_identity(nc, ident[:])
                ones1f = constp.tile([1, P], F32, tag="ones1f")
                nc.vector.memset(ones1f[:], 1.0)
                if ldw_opt:
                    # distinct BIR hash for the ldw-opt NEFF cache entry
                    nc.vector.memset(ones1f[0:1, 0:1], 1.0)
                ones1 = constp.tile([1, P], AUXDT, tag="ones1")
                nc.vector.tensor_copy(ones1[:], ones1f[:])
                sb1_sb = constp.tile([P, HC], F32, tag="sb1")
                nc.sync.dma_start(sb1_sb[:], sb1_t[:])
                sb2_sb = constp.tile([P, DC], F32, tag="sb2")
                nc.sync.dma_start(sb2_sb[:], sb2_t[:])
                rb1_sb = constp.tile([P, E, HC], F32, tag="rb1")
                nc.sync.dma_start(rb1_sb[:], rb1_t[:])
                rb2_sb = constp.tile([E, D], F32, tag="rb2")
                nc.sync.dma_start(rb2_sb[:], rb2[:])
                rb2_r = constp.tile([E, D], AUXDT, tag="rb2r")
                nc.vector.tensor_copy(rb2_r[:], rb2_sb[:])

                # ---------- x load + router logits (feature-major) + fp32r cast ----------
                NTCH = TC // P  # 8 token chunks of 128
                xr = xrp.tile([P, DC, TC], MMDT, tag="xr")
                ps_lf = pslp.tile([E, TC], F32, tag="psl")
                for dc in range(DC):
                    xc = xcp.tile([P, TC], F32, tag="xc")
                    nc.sync.dma_start(xc[:], xT_t[:, dc, :])
                    for tb in range(NTB):
                        nc.tensor.matmul(
                            ps_lf[:, tb * TB:(tb + 1) * TB],
                            gw_sb[:, dc, :],
                            xc[:, tb * TB:(tb + 1) * TB],
                            start=(dc == 0), stop=(dc == DC - 1),
                        )
                    nc.vector.tensor_copy(xr[:, dc, :], xc[:])

                # logits + expert bias (feature-major), then transpose to token-major
                lgT = rtp.tile([E, TC], F32, tag="lgT", name="lgT")
                nc.vector.tensor_scalar_add(lgT[:], ps_lf[:], eb_sb[:])

                # ---------- router (fp32, token-major [128, 8, 7]) ----------
                def rt3(tag):
                    return rtp.tile([P, NTCH, E], F32, tag=tag, name=tag)

                def rt1(tag):
                    return rtp.tile([P, NTCH, 1], F32, tag=tag, name=tag)

                def bc3(t):  # broadcast [P, NTCH, 1] -> [P, NTCH, E] view
                    return t[:].to_broadcast((P, NTCH, E))

                rev3 = rev_sb[:, None, :].to_broadcast((P, NTCH, E))

                lg = rt3("lg")
                for tch in range(NTCH):
                    pt = psp.tile([P, E], F32, tag="ps", name="pt")
                    nc.tensor.transpose(pt[:], lgT[:, tch * P:(tch + 1) * P], ident[0:E, 0:E])
                    nc.vector.tensor_copy(lg[:, tch, :], pt[:])
                m1 = rt1("m1")
                nc.vector.reduce_max(m1[:], lg[:], axis=mybir.AxisListType.X)
                mask1 = rt3("mask1")
                nc.vector.tensor_tensor(mask1[:], lg[:], bc3(m1), op=mybir.AluOpType.is_equal)
                mv1 = rt3("mv1")
                nc.vector.tensor_tensor(mv1[:], mask1[:], rev3, op=mybir.AluOpType.mult)
                sel1 = rt1("sel1")
                nc.vector.reduce_max(sel1[:], mv1[:], axis=mybir.AxisListType.X)
                m1f = rt3("m1f")
                nc.vector.tensor_tensor(m1f[:], mv1[:], bc3(sel1), op=mybir.AluOpType.is_equal)
                # l2 = lg - 1e30*mask1f
                l2 = rt3("l2")
                nc.vector.tensor_scalar(l2[:], m1f[:], -1.0e30, None, op0=mybir.AluOpType.mult)
                nc.vector.tensor_add(l2[:], l2[:], lg[:])
                m2 = rt1("m2")
                nc.vector.reduce_max(m2[:], l2[:], axis=mybir.AxisListType.X)
                mask2 = rt3("mask2")
                nc.vector.tensor_tensor(mask2[:], l2[:], bc3(m2), op=mybir.AluOpType.is_equal)
                mv2 = rt3("mv2")
                nc.vector.tensor_tensor(mv2[:], mask2[:], rev3, op=mybir.AluOpType.mult)
                sel2 = rt1("sel2")
                nc.vector.reduce_max(sel2[:], mv2[:], axis=mybir.AxisListType.X)
                m2f = rt3("m2f")
                nc.vector.tensor_tensor(m2f[:], mv2[:], bc3(sel2), op=mybir.AluOpType.is_equal)
                # softmax over (m1, m2): w1 = 1/(1+exp(m2-m1)); w2 = 1-w1
                dlt = rt1("dlt")
                nc.vector.tensor_sub(dlt[:], m2[:], m1[:])
                ex = rt1("ex")
                nc.scalar.activation(ex[:], dlt[:], mybir.ActivationFunctionType.Exp)
                den = rt1("den")
                nc.vector.tensor_scalar_add(den[:], ex[:], 1.0)
                w1t = rt1("w1t")
                nc.vector.reciprocal(w1t[:], den[:])
                w2t = rt1("w2t")
                nc.vector.tensor_mul(w2t[:], ex[:], w1t[:])
                comb3 = rt3("comb3")
                nc.vector.tensor_tensor(comb3[:], m1f[:], bc3(w1t), op=mybir.AluOpType.mult)
                m2fw = rt3("m2fw")
                nc.vector.tensor_tensor(m2fw[:], m2f[:], bc3(w2t), op=mybir.AluOpType.mult)
                nc.vector.tensor_add(comb3[:], comb3[:], m2fw[:])

                # transpose combine to feature-major [7, TC] via PE
                comb = rtp.tile([E, TC], F32, tag="comb", name="comb")
                for tch in range(NTCH):
                    pt2 = psp.tile([E, P], F32, tag="ps", name="pt2")
                    nc.tensor.transpose(pt2[:], comb3[:, tch, :], ident[:])
                    nc.vector.tensor_copy(comb[:, tch * P:(tch + 1) * P], pt2[:])

                comb_r = rtp.tile([E, TC], AUXDT, tag="combr", name="combr")
                nc.vector.tensor_copy(comb_r[:], comb[:])

                # combine rows -> single-partition layout (DMA is exempt from
                # partition-start rules), then broadcast across partitions via
                # a K=1 ones-matmul per (expert, token-block).
                bcs = bcp.tile([P, E, TC], F32, tag="bcs")
                for e in range(E):
                    combf = rtp.tile([1, TC], F32, tag="combf", name="combf", bufs=2)
                    nc.sync.dma_start(combf[0:1, :], comb[e:e + 1, :])
                    combfr = rtp.tile([1, TC], AUXDT, tag="combfr", name="combfr", bufs=2)
                    nc.vector.tensor_copy(combfr[:], combf[:])
                    for tb in range(NTB):
                        pb = psp.tile([P, TB], F32, tag="ps", name="pb")
                        nc.tensor.matmul(
                            pb[:], ones1[:],
                            combfr[0:1, tb * TB:(tb + 1) * TB],
                            start=True, stop=True,
                        )
                        nc.vector.tensor_copy(bcs[:, e, tb * TB:(tb + 1) * TB], pb[:])

                # ---------- networks: shared (n=0) then routed e=n-1 ----------
                out_acc = accp.tile([P, DC, TC], F32, tag="acc")
                WCH = 256  # h/d columns per weight DMA chunk (1 MiB)
                for n in range(1 + E):
                    w1v, w2v = w1_view(n), w2_view(n)
                    # L1: h = gelu(w1.T @ x + b1)   (feature-major: [H, TC])
                    hbuf = hp.tile([P, HC, TC], MMDT, tag="h")
                    for ci in range(H // WCH):
                        wt = wp.tile([P, DC, WCH], MMDT, tag="w")
                        nc.sync.dma_start(wt[:], w1v[:, :, ci * WCH:(ci + 1) * WCH])
                        for hl in range(WCH // P):
                            hc = ci * (WCH // P) + hl
                            ph = psp.tile([P, NTB, TB], F32, tag="ps")
                            for dc in range(DC):
                                for tb in range(NTB):
                                    nc.tensor.matmul(
                                        ph[:, tb, :],
                                        wt[:, dc, hl * P:(hl + 1) * P],
                                        xr[:, dc, tb * TB:(tb + 1) * TB],
                                        start=(dc == 0), stop=(dc == DC - 1),
                                    )
                            bias = sb1_sb[:, hc:hc + 1] if n == 0 else rb1_sb[:, n - 1, hc:hc + 1]
                            nc.scalar.activation(hbuf[:, hc, :], ph[:, :, :].rearrange("p a b -> p (a b)"),
                                                 act_fn, bias=bias)
                            if n > 0:
                                nc.vector.tensor_mul(hbuf[:, hc, :], hbuf[:, hc, :], bcs[:, n - 1, :])
                    # L2: out_acc += w2.T @ h (+ sb2 + sum_e c_e*rb2[e] via shared pass)
                    for ci in range(D // WCH):
                        wt2 = wp.tile([P, HC, WCH], MMDT, tag="w")
                        nc.sync.dma_start(wt2[:], w2v[:, :, ci * WCH:(ci + 1) * WCH])
                        for dl in range(WCH // P):
                            dc = ci * (WCH // P) + dl
                            po = psp.tile([P, NTB, TB], F32, tag="ps")
                            for hc in range(HC):
                                for tb in range(NTB):
                                    nc.tensor.matmul(
                                        po[:, tb, :],
                                        wt2[:, hc, dl * P:(dl + 1) * P],
                                        hbuf[:, hc, tb * TB:(tb + 1) * TB],
                                        start=(hc == 0), stop=(hc == HC - 1 and n > 0),
                                    )
                            if n == 0:
                                # fold in sum_e combine_e * rb2[e] (K=7 matmul), then bias sb2
                                for tb in range(NTB):
                                    nc.tensor.matmul(
                                        po[:, tb, :],
                                        rb2_r[:, dc * P:(dc + 1) * P],
                                        comb_r[:, tb * TB:(tb + 1) * TB],
                                        start=False, stop=True,
                                    )
                                nc.vector.tensor_scalar_add(
                                    out_acc[:, dc, :], po[:].rearrange("p a b -> p (a b)"),
                                    sb2_sb[:, dc:dc + 1])
                            else:
                                nc.vector.tensor_add(
                                    out_acc[:, dc, :], out_acc[:, dc, :],
                                    po[:].rearrange("p a b -> p (a b)"))

                nc.sync.dma_start(outT_t[:], out_acc[:])

            if n_reps == 1:
                body()
            else:
                tc.For_i_unrolled(0, n_reps, 1, body, max_unroll=1)

    nc.compile()
    return nc


_NC_CACHE = {}


def _get_nc(n_reps=1):
    if n_reps not in _NC_CACHE:
        _NC_CACHE[n_reps] = build_nc(n_reps)
    return _NC_CACHE[n_reps]


class Runner:
    """Compile once, dispatch many times (axon/PJRT path). Inputs are the
    concatenated per-core arrays (axis 0); outputs likewise."""

    def __init__(self, nc):
        import jax
        from jax.sharding import Mesh, PartitionSpec
        from jax.experimental.shard_map import shard_map
        from concourse import bass2jax

        bass2jax.install_neuronx_cc_hook()
        self.nc = nc
        self.jax = jax
        pname = nc.partition_id_tensor.name if nc.partition_id_tensor else None
        in_names, out_names, out_avals = [], [], []
        for alloc in nc.m.functions[0].allocations:
            if not isinstance(alloc, mybir.MemoryLocationSet):
                continue
            name = alloc.memorylocations[0].name
            if alloc.kind == "ExternalInput":
                if name != pname:
                    in_names.append(name)
            elif alloc.kind == "ExternalOutput":
                out_names.append(name)
                out_avals.append(jax.core.ShapedArray(
                    tuple(alloc.tensor_shape), mybir.dt.np(alloc.dtype)))
        self.in_names, self.out_names, self.out_avals = in_names, out_names, out_avals
        all_names = in_names + out_names + ([pname] if pname else [])

        def _body(*args):
            operands = list(args)
            if pname is not None:
                operands.append(bass2jax.partition_id_tensor())
            outs = bass2jax._bass_exec_p.bind(
                *operands,
                out_avals=tuple(out_avals),
                in_names=tuple(all_names),
                out_names=tuple(out_names),
                lowering_input_output_aliases=(),
                sim_require_finite=True, sim_require_nnan=True, nc=nc)
            return tuple(outs)

        devices = jax.devices()[:N_CORES]
        mesh = Mesh(np.asarray(devices), ("core",))
        nin = len(in_names) + len(out_names)
        self.fn = jax.jit(
            shard_map(_body, mesh=mesh,
                      in_specs=(PartitionSpec("core"),) * nin,
                      out_specs=(PartitionSpec("core"),) * len(out_names),
                      check_rep=False),
            keep_unused=True)

    def concat_inputs(self, in_maps):
        args = []
        for name in self.in_names:
            args.append(np.concatenate([m[name] for m in in_maps], axis=0))
        for av in self.out_avals:
            args.append(np.zeros((N_CORES * av.shape[0],) + av.shape[1:], av.dtype))
        return args

    def __call__(self, args):
        outs = self.fn(*args)
        self.jax.block_until_ready(outs)
        return outs

    def split_outputs(self, outs):
        res = []
        for c in range(N_CORES):
            d = {}
            for i, name in enumerate(self.out_names):
                a = np.asarray(outs[i])
                d[name] = a.reshape(N_CORES, *self.out_avals[i].shape)[c]
            res.append(d)
        return res


_RUNNER_CACHE = {}


def get_runner(n_reps=1, act_fn=None, mm_mode="f32r", aux_f32r=True, ldw_opt=True):
    key = (n_reps, act_fn, mm_mode, aux_f32r, ldw_opt)
    if key not in _RUNNER_CACHE:
        global _LDW_OPT
        prev = _LDW_OPT
        _LDW_OPT = ldw_opt
        try:
            r = Runner(build_nc(n_reps, act_fn=act_fn, mm_mode=mm_mode,
                                aux_f32r=aux_f32r, ldw_opt=ldw_opt))
            # force compile now, while the flag is set
            r(r.concat_inputs(_dummy_in_maps(r)))
        finally:
            _LDW_OPT = prev
        _RUNNER_CACHE[key] = r
    return _RUNNER_CACHE[key]


def _dummy_in_maps(runner):
    maps = []
    for c in range(N_CORES):
        m = {}
        for alloc in runner.nc.m.functions[0].allocations:
            if not isinstance(alloc, mybir.MemoryLocationSet):
                continue
            name = alloc.memorylocations[0].name
            if alloc.kind == "ExternalInput" and name in runner.in_names:
                m[name] = np.zeros(tuple(alloc.tensor_shape),
                                   mybir.dt.np(alloc.dtype))
        maps.append(m)
    return maps


def make_in_maps(x, gate_w, expert_bias, sw1, sb1, sw2, sb2, rw1, rb1, rw2, rb2,
                 mm_mode: str = "f32r"):
    wdt = {"f32r": np.float32, "f16": np.float16, "bf16": None}[mm_mode]
    xf = np.ascontiguousarray(np.asarray(x, dtype=np.float32).reshape(T, D))
    gwT = np.ascontiguousarray(np.asarray(gate_w, np.float32).T)
    rev = np.arange(E, 0, -1, dtype=np.float32)

    def w(a):
        return np.ascontiguousarray(np.asarray(a, np.float32).astype(wdt))

    shared = {
        "gwT": gwT,
        "eb": np.ascontiguousarray(np.asarray(expert_bias, np.float32)),
        "rev7": rev,
        "sw1": w(sw1),
        "sb1": np.ascontiguousarray(np.asarray(sb1, np.float32)),
        "sw2": w(sw2),
        "sb2": np.ascontiguousarray(np.asarray(sb2, np.float32)),
        "rw1": w(rw1),
        "rb1": np.ascontiguousarray(np.asarray(rb1, np.float32)),
        "rw2": w(rw2),
        "rb2": np.ascontiguousarray(np.asarray(rb2, np.float32)),
    }
    in_maps = []
    for c in range(N_CORES):
        xTc = np.ascontiguousarray(xf[c * TC:(c + 1) * TC, :].T)
        in_maps.append({"xT": xTc, **shared})
    return in_maps


def kernel(x, gate_w, expert_bias, sw1, sb1, sw2, sb2, rw1, rb1, rw2, rb2):
    runner = get_runner(1)
    in_maps = make_in_maps(x, gate_w, expert_bias, sw1, sb1, sw2, sb2,
                           rw1, rb1, rw2, rb2)
    outs = runner(runner.concat_inputs(in_maps))
    res = runner.split_outputs(outs)
    parts = [res[c]["outT"].T for c in range(N_CORES)]
    out = np.concatenate(parts, axis=0).reshape(B, S, D)
    return np.ascontiguousarray(out.astype(np.float32))

